# revision 9
# baseline (speedup 1.0000x reference)
"""Causal self-attention TRN2 Bass kernel.

Sharding: 8 cores = 4 batches x 2 head-groups. Core c handles batch c//2 and
heads (c%2)*8 .. (c%2)*8+8 (of 16). Each core computes its heads' attention
and a partial output projection; the host sums the two partials per batch and
adds b_out.

All matmuls run in float32r (fp32 storage, reduced-mantissa multiplies at full
PE rate). Intermediates accumulate in fp32 PSUM.

Layouts on chip (per core):
  xT   [1024, 2048]  x[b].T, host-pretransposed
  QT,KT [512, 2048]  per-head-group q/k features x tokens (8 tiles [128,2048])
  V    [2048, 520]   tokens x (8 heads x (64 vals + ones col))
  S^T  [k, q] tiles  -> exp -> PV^T accumulation gives [65, q] per head
                       (row 64 = softmax denominator via the ones column)
  AoT  [512, 2048]   normalized attention output (features x tokens)
  y    [2048, 1024]  partial output projection (natural layout)
"""
import sys

sys.path.insert(0, "/opt/trn_rl_repo")

import numpy as np

D_MODEL = 1024
N_HEADS = 16
B = 4
T = 2048
HD = 64
N_CORES = 8
NH_LOC = N_HEADS // 2  # heads per core
FQ = NH_LOC * HD  # 512 local features

_prog_cache = {}


def build_program(tok=T):
    """Build the single-core SPMD Bass program. tok must be a multiple of 512."""
    import concourse.mybir as mybir
    import concourse.tile as tile
    from concourse import bacc

    f32 = mybir.dt.float32
    bf16 = mybir.dt.bfloat16
    f32r = mybir.dt.float32r
    P = 128
    QC = 512  # q-chunk width
    KC = D_MODEL // P  # 8 d-model chunks
    TT = tok // P  # token tiles
    NJ = tok // QC  # q-chunks
    NDC = FQ // P  # 4 din chunks

    nc = bacc.Bacc("TRN2", target_bir_lowering=False, debug=False, num_devices=N_CORES)

    xT = nc.dram_tensor("xT", [D_MODEL, tok], f32r, kind="ExternalInput")
    wq = nc.dram_tensor("wq", [D_MODEL, FQ], f32r, kind="ExternalInput")
    wk = nc.dram_tensor("wk", [D_MODEL, FQ], f32r, kind="ExternalInput")
    wv = nc.dram_tensor("wv", [D_MODEL, FQ], f32r, kind="ExternalInput")
    wo = nc.dram_tensor("wo", [FQ, D_MODEL], f32r, kind="ExternalInput")
    y = nc.dram_tensor("y", [tok, D_MODEL], f32, kind="ExternalOutput")

    with tile.TileContext(nc) as tc:
        with (
            tc.tile_pool(name="qkt", bufs=1) as qktp,
            tc.tile_pool(name="vp", bufs=1) as vp,
            tc.tile_pool(name="mask", bufs=1) as maskp,
            tc.tile_pool(name="nrmo", bufs=1) as nrmo,
            # shared PSUM pools for all phases: no pool-transition barrier,
            # so the PE never sees a multi-us gap (keeps HAM at 8/8)
            tc.tile_pool(name="big", bufs=2, space="PSUM") as bigp,
            tc.tile_pool(name="small", bufs=4, space="PSUM") as smallp,
        ):
            # persistent tiles
            QKT = [qktp.tile([P, tok], f32r, tag=f"qkt{i}", name=f"qkt{i}") for i in range(8)]
            V = [vp.tile([P, NH_LOC * (HD + 1)], f32r, tag=f"v{i}", name=f"v{i}") for i in range(TT)]

            # single causal mask triangle, duplicated for the two heads:
            # mask[p, q] = 0 where q >= p else -1e30, shape [128, 2*128]
            cmask = maskp.tile([P, 2 * P], bf16, tag="cmask", name="cmask")
            nc.gpsimd.memset(cmask[:], 0.0)
            for half in (0, 1):
                nc.gpsimd.affine_select(
                    out=cmask[:, half * P : (half + 1) * P],
                    in_=cmask[:, half * P : (half + 1) * P],
                    compare_op=mybir.AluOpType.is_ge,
                    fill=-1e30,
                    base=0,
                    pattern=[[1, P]],
                    channel_multiplier=-1,
                )

            # ---------------- Phase 1: projections ----------------
            with (
                tc.tile_pool(name="xt", bufs=1) as xtp,
                tc.tile_pool(name="wst", bufs=2) as wp,
                tc.tile_pool(name="wvp", bufs=1) as wvp,
            ):
                XT = [xtp.tile([P, tok], f32r, tag=f"xt{l}", name=f"xt{l}") for l in range(KC)]
                WV = [wvp.tile([P, FQ], f32r, tag=f"wv{l}", name=f"wv{l}") for l in range(KC)]
                WSL = {}
                # DMA issue order matters on the sync queue: the first two
                # weight slices come first so the PE can start immediately
                # after the first xT tiles land
                for ft in (0, 1):
                    WSL[ft] = wp.tile([P, KC * P], f32r, tag="w", name=f"wsl{ft}")
                    nc.sync.dma_start(
                        out=WSL[ft][:].rearrange("p (l f) -> p l f", f=P),
                        in_=wq[:, ft * P : (ft + 1) * P].rearrange(
                            "(l p) f -> p l f", p=P
                        ),
                    )
                for cc in range(tok // QC):
                    for l in range(KC):
                        nc.sync.dma_start(
                            out=XT[l][:, cc * QC : (cc + 1) * QC],
                            in_=xT[l * P : (l + 1) * P, cc * QC : (cc + 1) * QC],
                        )
                for l in range(KC):
                    nc.sync.dma_start(out=WV[l][:], in_=wv[l * P : (l + 1) * P, :])

                def vproj_block(tt):
                    nc.gpsimd.memset(V[tt][:].bitcast(f32), 1.0)
                    psv = smallp.tile([P, FQ], f32, tag="small", name="psv")
                    for l in range(KC):
                        nc.tensor.matmul(
                            psv[:],
                            XT[l][:, tt * P : (tt + 1) * P],
                            WV[l][:],
                            start=(l == 0),
                            stop=(l == KC - 1),
                        )
                    # strided copy into the 65-col head groups; col 64 of
                    # each group keeps the memset 1.0 (=> PV row 64 = denom)
                    vdst = V[tt][:].rearrange("p (u c) -> p u c", c=HD + 1)[
                        :, :, 0:HD
                    ]
                    vsrc = psv[:].rearrange("p (u c) -> p u c", c=HD)
                    nc.scalar.copy(vdst, vsrc)

                # Q^T and K^T interleaved with V tiles: V blocks fill the PE
                # while a ft block waits for its PSUM accumulators to drain
                vq = iter(range(TT))
                for ft in range(8):
                    wsrc = wq if ft < 4 else wk
                    fo = (ft % 4) * P
                    if ft in WSL:
                        wsl = WSL[ft]
                    else:
                        wsl = wp.tile([P, KC * P], f32r, tag="w", name=f"wsl{ft}")
                        nc.sync.dma_start(
                            out=wsl[:].rearrange("p (l f) -> p l f", f=P),
                            in_=wsrc[:, fo : fo + P].rearrange("(l p) f -> p l f", p=P),
                        )
                    qs = list(range(NJ))
                    pbs = [
                        bigp.tile([P, 2 * QC], f32, tag="big", name="pqk")
                        for _ in range((NJ + 1) // 2)
                    ]
                    half = {
                        q: pbs[q // 2][:, (q % 2) * QC : (q % 2 + 1) * QC] for q in qs
                    }
                    for l in range(KC):
                        for q in qs:
                            nc.tensor.matmul(
                                half[q],
                                wsl[:, l * P : (l + 1) * P],
                                XT[l][:, q * QC : (q + 1) * QC],
                                start=(l == 0),
                                stop=(l == KC - 1),
                            )
                    for q in qs:
                        nc.scalar.copy(QKT[ft][:, q * QC : (q + 1) * QC], half[q])
                    if ft >= 1:
                        for _ in range(2 if ft < 7 else 4):
                            tt = next(vq, None)
                            if tt is not None:
                                vproj_block(tt)
                for tt in vq:
                    vproj_block(tt)

            # ---------------- Phase 2: causal attention ----------------
            with (
                tc.tile_pool(name="aot", bufs=1) as aotp,
                tc.tile_pool(name="exp", bufs=4) as expp,
                tc.tile_pool(name="wop", bufs=1) as wop,
                tc.tile_pool(name="yp", bufs=4) as yp,
            ):
                AOT = [aotp.tile([P, tok], f32r, tag=f"aot{d}", name=f"aot{d}") for d in range(NDC)]
                WO = [
                    wop.tile([P, D_MODEL], f32r, tag=f"wo{d}", name=f"wo{d}") for d in range(NDC)
                ]
                for d in range(NDC):
                    nc.sync.dma_start(out=WO[d][:], in_=wo[d * P : (d + 1) * P, :])

                for j in range(NJ):
                    for hp in range(NH_LOC // 2):
                        nkt = 4 * j + 4  # k-tiles for this q-chunk
                        uA, uB = 2 * hp, 2 * hp + 1
                        us = (uA, uB)
                        pv = {u: smallp.tile([HD + 1, QC], f32, tag="small", name=f"pv{u}") for u in us}
                        for i in range(nkt):
                            # one k-tile per step; both heads packed into one
                            # [128, 1024] PSUM tile (A: cols 0:512, B: 512:1024)
                            # -> 3 steps in flight with bufs=3, and the two
                            # 64-row ST matmuls alternate PE row groups.
                            # Diagonal k-tiles (s = i - 4j >= 0) only touch
                            # q >= 128*s, so all work shrinks to the window
                            # [128*s : 512] and the mask-add reduces to one
                            # shared 128-wide causal triangle at the window
                            # start.
                            s = i - 4 * j
                            w0 = 128 * s if s >= 0 else 0
                            wn = QC - w0
                            st = bigp.tile([P, 2 * QC], f32, tag="big", name="st")
                            for idx, u in enumerate(us):
                                rs = slice(64 * (u % 2), 64 * (u % 2) + 64)
                                nc.tensor.matmul(
                                    st[:, idx * QC + w0 : (idx + 1) * QC],
                                    QKT[4 + u // 2][rs, i * P : (i + 1) * P],
                                    QKT[u // 2][rs, j * QC + w0 : (j + 1) * QC],
                                    start=True,
                                    stop=True,
                                )
                            win3 = st[:].rearrange("p (h q) -> p h q", h=2)
                            if s >= 0:
                                nc.vector.tensor_tensor(
                                    out=win3[:, :, w0 : w0 + P],
                                    in0=win3[:, :, w0 : w0 + P],
                                    in1=cmask[:].rearrange(
                                        "p (h q) -> p h q", h=2
                                    ),
                                    op=mybir.AluOpType.add,
                                )
                            e = expp.tile([P, 2 * QC], f32r, tag="e", name="e")
                            nc.scalar.activation(
                                e[:].rearrange("p (h q) -> p h q", h=2)[
                                    :, :, w0:QC
                                ],
                                win3[:, :, w0:QC],
                                mybir.ActivationFunctionType.Exp,
                                scale=0.125,
                            )
                            for idx, u in enumerate(us):
                                nc.tensor.matmul(
                                    pv[u][:, w0:QC],
                                    V[i][:, u * (HD + 1) : (u + 1) * (HD + 1)],
                                    e[:, idx * QC + w0 : (idx + 1) * QC],
                                    start=(i == 0),
                                    stop=(i == nkt - 1),
                                )
                        for u in us:
                            # row 64 of pv = denominator; broadcast it, fast
                            # reciprocal on 64 lanes, then normalize
                            sd = nrmo.tile([1, QC], f32, tag="sd", name="sd")
                            nc.vector.tensor_copy(sd[:], pv[u][HD : HD + 1, :])
                            bc = nrmo.tile([HD, QC], f32, tag="bc", name="bc")
                            nc.gpsimd.partition_broadcast(bc[:], sd[:])
                            nc.vector.reciprocal_approx_fast(bc[:], bc[:])
                            nc.vector.tensor_tensor(
                                out=AOT[u // 2][
                                    64 * (u % 2) : 64 * (u % 2) + 64,
                                    j * QC : (j + 1) * QC,
                                ],
                                in0=pv[u][0:HD, :],
                                in1=bc[:],
                                op=mybir.AluOpType.mult,
                            )

                # ---------------- Phase 3: output projection ----------------
                for tt in range(TT):
                    pb = bigp.tile([P, 2 * QC], f32, tag="big", name="py")
                    for d in range(NDC):
                        for h in (0, 1):
                            nc.tensor.matmul(
                                pb[:, h * QC : (h + 1) * QC],
                                AOT[d][:, tt * P : (tt + 1) * P],
                                WO[d][:, h * QC : (h + 1) * QC],
                                start=(d == 0),
                                stop=(d == NDC - 1),
                            )
                    for h in (0, 1):
                        ysb = yp.tile([P, QC], f32, tag="y")
                        nc.scalar.copy(ysb[:], pb[:, h * QC : (h + 1) * QC])
                        nc.sync.dma_start(
                            out=y[tt * P : (tt + 1) * P, h * QC : (h + 1) * QC],
                            in_=ysb[:],
                        )
    nc.compile()
    return nc


def get_program(tok=T):
    if tok not in _prog_cache:
        _prog_cache[tok] = build_program(tok)
    return _prog_cache[tok]


def make_in_maps(x, w_qkv, w_out):
    """Shard full inputs into 8 per-core input maps."""
    x = np.asarray(x, dtype=np.float32)
    w_qkv = np.asarray(w_qkv, dtype=np.float32)
    w_out = np.asarray(w_out, dtype=np.float32)
    D = D_MODEL
    xTs = [np.ascontiguousarray(x[b].T) for b in range(x.shape[0])]
    in_maps = []
    for c in range(N_CORES):
        b, hg = c // 2, c % 2
        in_maps.append(
            {
                "xT": xTs[b],
                "wq": np.ascontiguousarray(w_qkv[:, hg * FQ : (hg + 1) * FQ]),
                "wk": np.ascontiguousarray(
                    w_qkv[:, D + hg * FQ : D + (hg + 1) * FQ]
                ),
                "wv": np.ascontiguousarray(
                    w_qkv[:, 2 * D + hg * FQ : 2 * D + (hg + 1) * FQ]
                ),
                "wo": np.ascontiguousarray(w_out[hg * FQ : (hg + 1) * FQ, :]),
            }
        )
    return in_maps


_runner_cache = {}


def _make_runner(nc, n_cores=N_CORES):
    """Cached multi-core executor (same semantics as bass2jax.run_bass_via_pjrt
    for a program with no partition-id and no debug tensors, but the jitted
    callable is reusable so repeat kernel() calls don't recompile)."""
    import jax
    from jax.sharding import Mesh, PartitionSpec
    from jax.experimental.shard_map import shard_map
    import concourse.mybir as mybir
    from concourse.bass2jax import _bass_exec_p, install_neuronx_cc_hook

    install_neuronx_cc_hook()

    in_names, out_names, out_avals = [], [], []
    for alloc in nc.m.functions[0].allocations:
        if not isinstance(alloc, mybir.MemoryLocationSet):
            continue
        name = alloc.memorylocations[0].name
        if alloc.kind == "ExternalInput":
            in_names.append(name)
        elif alloc.kind == "ExternalOutput":
            out_names.append(name)
            out_avals.append(
                jax.core.ShapedArray(
                    tuple(alloc.tensor_shape), mybir.dt.np(alloc.dtype)
                )
            )
    n_params = len(out_names) and len(in_names)
    n_params = len(in_names)
    n_outs = len(out_avals)
    all_in_names = in_names + out_names

    def _body(*args):
        outs = _bass_exec_p.bind(
            *args,
            out_avals=tuple(out_avals),
            in_names=tuple(all_in_names),
            out_names=tuple(out_names),
            lowering_input_output_aliases=(),
            sim_require_finite=True,
            sim_require_nnan=True,
            nc=nc,
        )
        return tuple(outs)

    devices = jax.devices()[:n_cores]
    mesh = Mesh(np.asarray(devices), ("core",))
    donate = tuple(range(n_params, n_params + n_outs))
    sharded = jax.jit(
        shard_map(
            _body,
            mesh=mesh,
            in_specs=(PartitionSpec("core"),) * (n_params + n_outs),
            out_specs=(PartitionSpec("core"),) * n_outs,
            check_rep=False,
        ),
        donate_argnums=donate,
        keep_unused=True,
    )

    def run(in_maps):
        per_core = [[np.asarray(m[nm]) for nm in in_names] for m in in_maps]
        concat_in = [
            np.concatenate([per_core[c][i] for c in range(n_cores)], axis=0)
            for i in range(n_params)
        ]
        concat_zeros = [
            np.zeros((n_cores * a.shape[0], *a.shape[1:]), a.dtype)
            for a in out_avals
        ]
        out_arrs = sharded(*concat_in, *concat_zeros)
        return [
            {
                nm: np.asarray(out_arrs[i]).reshape(n_cores, *out_avals[i].shape)[c]
                for i, nm in enumerate(out_names)
            }
            for c in range(n_cores)
        ]

    return run


def get_runner(tok=T):
    if tok not in _runner_cache:
        _runner_cache[tok] = _make_runner(get_program(tok))
    return _runner_cache[tok]


def kernel(x, w_qkv, w_out, b_out):
    run = get_runner(T)
    in_maps = make_in_maps(x, w_qkv, w_out)
    results = run(in_maps)
    b_out = np.asarray(b_out, dtype=np.float32)
    out = np.empty((B, T, D_MODEL), dtype=np.float32)
    for b in range(B):
        out[b] = results[2 * b]["y"] + results[2 * b + 1]["y"] + b_out
    return out


# revision 10
# speedup vs baseline: 1.0416x; 1.0416x over previous
"""Causal self-attention TRN2 Bass kernel.

Sharding: 8 cores = 4 batches x 2 head-groups. Core c handles batch c//2 and
heads (c%2)*8 .. (c%2)*8+8 (of 16). Each core computes its heads' attention
and a partial output projection; the host sums the two partials per batch and
adds b_out.

All matmuls run in float32r (fp32 storage, reduced-mantissa multiplies at full
PE rate). Intermediates accumulate in fp32 PSUM.

Layouts on chip (per core):
  xT   [1024, 2048]  x[b].T, host-pretransposed
  QT,KT [512, 2048]  per-head-group q/k features x tokens (8 tiles [128,2048])
  V    [2048, 520]   tokens x (8 heads x (64 vals + ones col))
  S^T  [k, q] tiles  -> exp -> PV^T accumulation gives [65, q] per head
                       (row 64 = softmax denominator via the ones column)
  AoT  [512, 2048]   normalized attention output (features x tokens)
  y    [2048, 1024]  partial output projection (natural layout)
"""
import sys

sys.path.insert(0, "/opt/trn_rl_repo")

import numpy as np

D_MODEL = 1024
N_HEADS = 16
B = 4
T = 2048
HD = 64
N_CORES = 8
NH_LOC = N_HEADS // 2  # heads per core
FQ = NH_LOC * HD  # 512 local features

_prog_cache = {}


def build_program(tok=T):
    """Build the single-core SPMD Bass program. tok must be a multiple of 512."""
    import concourse.mybir as mybir
    import concourse.tile as tile
    from concourse import bacc

    f32 = mybir.dt.float32
    bf16 = mybir.dt.bfloat16
    f32r = mybir.dt.float32r
    P = 128
    QC = 512  # q-chunk width
    KC = D_MODEL // P  # 8 d-model chunks
    TT = tok // P  # token tiles
    NJ = tok // QC  # q-chunks
    NDC = FQ // P  # 4 din chunks

    nc = bacc.Bacc("TRN2", target_bir_lowering=False, debug=False, num_devices=N_CORES)

    xT = nc.dram_tensor("xT", [D_MODEL, tok], f32r, kind="ExternalInput")
    wq = nc.dram_tensor("wq", [D_MODEL, FQ], f32r, kind="ExternalInput")
    wk = nc.dram_tensor("wk", [D_MODEL, FQ], f32r, kind="ExternalInput")
    wv = nc.dram_tensor("wv", [D_MODEL, FQ], f32r, kind="ExternalInput")
    wo = nc.dram_tensor("wo", [FQ, D_MODEL], f32r, kind="ExternalInput")
    y = nc.dram_tensor("y", [tok, D_MODEL], f32, kind="ExternalOutput")

    with tile.TileContext(nc) as tc:
        with (
            tc.tile_pool(name="qkt", bufs=1) as qktp,
            tc.tile_pool(name="vp", bufs=1) as vp,
            tc.tile_pool(name="mask", bufs=1) as maskp,
            tc.tile_pool(name="nrmo", bufs=1) as nrmo,
            # shared PSUM pools for all phases: no pool-transition barrier,
            # so the PE never sees a multi-us gap (keeps HAM at 8/8)
            tc.tile_pool(name="big", bufs=3, space="PSUM") as bigp,
            tc.tile_pool(name="small", bufs=2, space="PSUM") as smallp,
        ):
            # persistent tiles
            QKT = [qktp.tile([P, tok], f32r, tag=f"qkt{i}", name=f"qkt{i}") for i in range(8)]
            V = [vp.tile([P, NH_LOC * (HD + 1)], f32r, tag=f"v{i}", name=f"v{i}") for i in range(TT)]

            # single causal mask triangle, duplicated for the two heads:
            # mask[p, q] = 0 where q >= p else -1e30, shape [128, 2*128]
            cmask = maskp.tile([P, 2 * P], bf16, tag="cmask", name="cmask")
            nc.gpsimd.memset(cmask[:], 0.0)
            for half in (0, 1):
                nc.gpsimd.affine_select(
                    out=cmask[:, half * P : (half + 1) * P],
                    in_=cmask[:, half * P : (half + 1) * P],
                    compare_op=mybir.AluOpType.is_ge,
                    fill=-1e30,
                    base=0,
                    pattern=[[1, P]],
                    channel_multiplier=-1,
                )

            # ---------------- Phase 1: projections ----------------
            with (
                tc.tile_pool(name="xt", bufs=1) as xtp,
                tc.tile_pool(name="wst", bufs=2) as wp,
                tc.tile_pool(name="wvp", bufs=1) as wvp,
            ):
                XT = [xtp.tile([P, tok], f32r, tag=f"xt{l}", name=f"xt{l}") for l in range(KC)]
                WV = [wvp.tile([P, FQ], f32r, tag=f"wv{l}", name=f"wv{l}") for l in range(KC)]
                WSL = {}
                # DMA issue order matters on the sync queue: the first two
                # weight slices come first so the PE can start immediately
                # after the first xT tiles land
                for ft in (0, 1):
                    WSL[ft] = wp.tile([P, KC * P], f32r, tag="w", name=f"wsl{ft}")
                    nc.sync.dma_start(
                        out=WSL[ft][:].rearrange("p (l f) -> p l f", f=P),
                        in_=wq[:, ft * P : (ft + 1) * P].rearrange(
                            "(l p) f -> p l f", p=P
                        ),
                    )
                for l in range(KC):
                    nc.sync.dma_start(out=XT[l][:], in_=xT[l * P : (l + 1) * P, :])
                for l in range(KC):
                    nc.sync.dma_start(out=WV[l][:], in_=wv[l * P : (l + 1) * P, :])

                def vproj_block(tt):
                    nc.gpsimd.memset(V[tt][:].bitcast(f32), 1.0)
                    psv = smallp.tile([P, FQ], f32, tag="small", name="psv")
                    for l in range(KC):
                        nc.tensor.matmul(
                            psv[:],
                            XT[l][:, tt * P : (tt + 1) * P],
                            WV[l][:],
                            start=(l == 0),
                            stop=(l == KC - 1),
                        )
                    # strided copy into the 65-col head groups; col 64 of
                    # each group keeps the memset 1.0 (=> PV row 64 = denom)
                    vdst = V[tt][:].rearrange("p (u c) -> p u c", c=HD + 1)[
                        :, :, 0:HD
                    ]
                    vsrc = psv[:].rearrange("p (u c) -> p u c", c=HD)
                    nc.scalar.copy(vdst, vsrc)

                # Q^T and K^T interleaved with V tiles: V blocks fill the PE
                # while a ft block waits for its PSUM accumulators to drain
                vq = iter(range(TT))
                for ft in range(8):
                    wsrc = wq if ft < 4 else wk
                    fo = (ft % 4) * P
                    if ft in WSL:
                        wsl = WSL[ft]
                    else:
                        wsl = wp.tile([P, KC * P], f32r, tag="w", name=f"wsl{ft}")
                        nc.sync.dma_start(
                            out=wsl[:].rearrange("p (l f) -> p l f", f=P),
                            in_=wsrc[:, fo : fo + P].rearrange("(l p) f -> p l f", p=P),
                        )
                    qs = list(range(NJ))
                    pbs = [
                        bigp.tile([P, 2 * QC], f32, tag="big", name="pqk")
                        for _ in range((NJ + 1) // 2)
                    ]
                    half = {
                        q: pbs[q // 2][:, (q % 2) * QC : (q % 2 + 1) * QC] for q in qs
                    }
                    for l in range(KC):
                        for q in qs:
                            nc.tensor.matmul(
                                half[q],
                                wsl[:, l * P : (l + 1) * P],
                                XT[l][:, q * QC : (q + 1) * QC],
                                start=(l == 0),
                                stop=(l == KC - 1),
                            )
                    for q in qs:
                        nc.scalar.copy(QKT[ft][:, q * QC : (q + 1) * QC], half[q])
                    if ft >= 1:
                        for _ in range(2 if ft < 7 else 4):
                            tt = next(vq, None)
                            if tt is not None:
                                vproj_block(tt)
                for tt in vq:
                    vproj_block(tt)

            # ---------------- Phase 2: causal attention ----------------
            with (
                tc.tile_pool(name="aot", bufs=1) as aotp,
                tc.tile_pool(name="exp", bufs=4) as expp,
                tc.tile_pool(name="wop", bufs=1) as wop,
                tc.tile_pool(name="yp", bufs=4) as yp,
            ):
                AOT = [aotp.tile([P, tok], f32r, tag=f"aot{d}", name=f"aot{d}") for d in range(NDC)]
                WO = [
                    wop.tile([P, D_MODEL], f32r, tag=f"wo{d}", name=f"wo{d}") for d in range(NDC)
                ]
                for d in range(NDC):
                    nc.sync.dma_start(out=WO[d][:], in_=wo[d * P : (d + 1) * P, :])

                for j in range(NJ):
                    for hp in range(NH_LOC // 2):
                        nkt = 4 * j + 4  # k-tiles for this q-chunk
                        uA, uB = 2 * hp, 2 * hp + 1
                        us = (uA, uB)
                        pv = {u: smallp.tile([HD + 1, QC], f32, tag="small", name=f"pv{u}") for u in us}
                        for i in range(nkt):
                            # one k-tile per step; both heads packed into one
                            # [128, 1024] PSUM tile (A: cols 0:512, B: 512:1024)
                            # -> 3 steps in flight with bufs=3, and the two
                            # 64-row ST matmuls alternate PE row groups.
                            # Diagonal k-tiles (s = i - 4j >= 0) only touch
                            # q >= 128*s, so all work shrinks to the window
                            # [128*s : 512] and the mask-add reduces to one
                            # shared 128-wide causal triangle at the window
                            # start.
                            s = i - 4 * j
                            w0 = 128 * s if s >= 0 else 0
                            wn = QC - w0
                            st = bigp.tile([P, 2 * QC], f32, tag="big", name="st")
                            for idx, u in enumerate(us):
                                rs = slice(64 * (u % 2), 64 * (u % 2) + 64)
                                nc.tensor.matmul(
                                    st[:, idx * QC + w0 : (idx + 1) * QC],
                                    QKT[4 + u // 2][rs, i * P : (i + 1) * P],
                                    QKT[u // 2][rs, j * QC + w0 : (j + 1) * QC],
                                    start=True,
                                    stop=True,
                                )
                            win3 = st[:].rearrange("p (h q) -> p h q", h=2)
                            if s >= 0:
                                nc.vector.tensor_tensor(
                                    out=win3[:, :, w0 : w0 + P],
                                    in0=win3[:, :, w0 : w0 + P],
                                    in1=cmask[:].rearrange(
                                        "p (h q) -> p h q", h=2
                                    ),
                                    op=mybir.AluOpType.add,
                                )
                            e = expp.tile([P, 2 * QC], f32r, tag="e", name="e")
                            nc.scalar.activation(
                                e[:].rearrange("p (h q) -> p h q", h=2)[
                                    :, :, w0:QC
                                ],
                                win3[:, :, w0:QC],
                                mybir.ActivationFunctionType.Exp,
                                scale=0.125,
                            )
                            for idx, u in enumerate(us):
                                nc.tensor.matmul(
                                    pv[u][:, w0:QC],
                                    V[i][:, u * (HD + 1) : (u + 1) * (HD + 1)],
                                    e[:, idx * QC + w0 : (idx + 1) * QC],
                                    start=(i == 0),
                                    stop=(i == nkt - 1),
                                )
                        for u in us:
                            # copy pv out fast (frees the slot), broadcast the
                            # denominator row, fast reciprocal, normalize
                            sa = nrmo.tile([HD, QC], f32, tag="sa", name="sa")
                            nc.vector.tensor_copy(sa[:], pv[u][0:HD, :])
                            sd = nrmo.tile([1, QC], f32, tag="sd", name="sd")
                            nc.vector.tensor_copy(sd[:], pv[u][HD : HD + 1, :])
                            bc = nrmo.tile([HD, QC], f32, tag="bc", name="bc")
                            nc.gpsimd.partition_broadcast(bc[:], sd[:])
                            nc.vector.reciprocal_approx_fast(bc[:], bc[:])
                            nc.vector.tensor_tensor(
                                out=AOT[u // 2][
                                    64 * (u % 2) : 64 * (u % 2) + 64,
                                    j * QC : (j + 1) * QC,
                                ],
                                in0=sa[:],
                                in1=bc[:],
                                op=mybir.AluOpType.mult,
                            )

                # ---------------- Phase 3: output projection ----------------
                for tt in range(TT):
                    pb = bigp.tile([P, 2 * QC], f32, tag="big", name="py")
                    for d in range(NDC):
                        for h in (0, 1):
                            nc.tensor.matmul(
                                pb[:, h * QC : (h + 1) * QC],
                                AOT[d][:, tt * P : (tt + 1) * P],
                                WO[d][:, h * QC : (h + 1) * QC],
                                start=(d == 0),
                                stop=(d == NDC - 1),
                            )
                    for h in (0, 1):
                        ysb = yp.tile([P, QC], f32, tag="y")
                        nc.scalar.copy(ysb[:], pb[:, h * QC : (h + 1) * QC])
                        nc.sync.dma_start(
                            out=y[tt * P : (tt + 1) * P, h * QC : (h + 1) * QC],
                            in_=ysb[:],
                        )
    nc.compile()
    return nc


def get_program(tok=T):
    if tok not in _prog_cache:
        _prog_cache[tok] = build_program(tok)
    return _prog_cache[tok]


def make_in_maps(x, w_qkv, w_out):
    """Shard full inputs into 8 per-core input maps."""
    x = np.asarray(x, dtype=np.float32)
    w_qkv = np.asarray(w_qkv, dtype=np.float32)
    w_out = np.asarray(w_out, dtype=np.float32)
    D = D_MODEL
    xTs = [np.ascontiguousarray(x[b].T) for b in range(x.shape[0])]
    in_maps = []
    for c in range(N_CORES):
        b, hg = c // 2, c % 2
        in_maps.append(
            {
                "xT": xTs[b],
                "wq": np.ascontiguousarray(w_qkv[:, hg * FQ : (hg + 1) * FQ]),
                "wk": np.ascontiguousarray(
                    w_qkv[:, D + hg * FQ : D + (hg + 1) * FQ]
                ),
                "wv": np.ascontiguousarray(
                    w_qkv[:, 2 * D + hg * FQ : 2 * D + (hg + 1) * FQ]
                ),
                "wo": np.ascontiguousarray(w_out[hg * FQ : (hg + 1) * FQ, :]),
            }
        )
    return in_maps


_runner_cache = {}


def _make_runner(nc, n_cores=N_CORES):
    """Cached multi-core executor (same semantics as bass2jax.run_bass_via_pjrt
    for a program with no partition-id and no debug tensors, but the jitted
    callable is reusable so repeat kernel() calls don't recompile)."""
    import jax
    from jax.sharding import Mesh, PartitionSpec
    from jax.experimental.shard_map import shard_map
    import concourse.mybir as mybir
    from concourse.bass2jax import _bass_exec_p, install_neuronx_cc_hook

    install_neuronx_cc_hook()

    in_names, out_names, out_avals = [], [], []
    for alloc in nc.m.functions[0].allocations:
        if not isinstance(alloc, mybir.MemoryLocationSet):
            continue
        name = alloc.memorylocations[0].name
        if alloc.kind == "ExternalInput":
            in_names.append(name)
        elif alloc.kind == "ExternalOutput":
            out_names.append(name)
            out_avals.append(
                jax.core.ShapedArray(
                    tuple(alloc.tensor_shape), mybir.dt.np(alloc.dtype)
                )
            )
    n_params = len(out_names) and len(in_names)
    n_params = len(in_names)
    n_outs = len(out_avals)
    all_in_names = in_names + out_names

    def _body(*args):
        outs = _bass_exec_p.bind(
            *args,
            out_avals=tuple(out_avals),
            in_names=tuple(all_in_names),
            out_names=tuple(out_names),
            lowering_input_output_aliases=(),
            sim_require_finite=True,
            sim_require_nnan=True,
            nc=nc,
        )
        return tuple(outs)

    devices = jax.devices()[:n_cores]
    mesh = Mesh(np.asarray(devices), ("core",))
    donate = tuple(range(n_params, n_params + n_outs))
    sharded = jax.jit(
        shard_map(
            _body,
            mesh=mesh,
            in_specs=(PartitionSpec("core"),) * (n_params + n_outs),
            out_specs=(PartitionSpec("core"),) * n_outs,
            check_rep=False,
        ),
        donate_argnums=donate,
        keep_unused=True,
    )

    def run(in_maps):
        per_core = [[np.asarray(m[nm]) for nm in in_names] for m in in_maps]
        concat_in = [
            np.concatenate([per_core[c][i] for c in range(n_cores)], axis=0)
            for i in range(n_params)
        ]
        concat_zeros = [
            np.zeros((n_cores * a.shape[0], *a.shape[1:]), a.dtype)
            for a in out_avals
        ]
        out_arrs = sharded(*concat_in, *concat_zeros)
        return [
            {
                nm: np.asarray(out_arrs[i]).reshape(n_cores, *out_avals[i].shape)[c]
                for i, nm in enumerate(out_names)
            }
            for c in range(n_cores)
        ]

    return run


def get_runner(tok=T):
    if tok not in _runner_cache:
        _runner_cache[tok] = _make_runner(get_program(tok))
    return _runner_cache[tok]


def kernel(x, w_qkv, w_out, b_out):
    run = get_runner(T)
    in_maps = make_in_maps(x, w_qkv, w_out)
    results = run(in_maps)
    b_out = np.asarray(b_out, dtype=np.float32)
    out = np.empty((B, T, D_MODEL), dtype=np.float32)
    for b in range(B):
        out[b] = results[2 * b]["y"] + results[2 * b + 1]["y"] + b_out
    return out


# revision 11
# speedup vs baseline: 1.1735x; 1.1266x over previous
"""Causal self-attention TRN2 Bass kernel.

Sharding: 8 cores = 4 batches x 2 head-groups. Core c handles batch c//2 and
heads (c%2)*8 .. (c%2)*8+8 (of 16). Each core computes its heads' attention
and a partial output projection; the host sums the two partials per batch and
adds b_out.

All matmuls run in float32r (fp32 storage, reduced-mantissa multiplies at full
PE rate). Intermediates accumulate in fp32 PSUM.

Layouts on chip (per core):
  xT   [1024, 2048]  x[b].T, host-pretransposed
  QT,KT [512, 2048]  per-head-group q/k features x tokens (8 tiles [128,2048])
  V    [2048, 520]   tokens x (8 heads x (64 vals + ones col))
  S^T  [k, q] tiles  -> exp -> PV^T accumulation gives [65, q] per head
                       (row 64 = softmax denominator via the ones column)
  AoT  [512, 2048]   normalized attention output (features x tokens)
  y    [2048, 1024]  partial output projection (natural layout)
"""
import sys

sys.path.insert(0, "/opt/trn_rl_repo")

import numpy as np

D_MODEL = 1024
N_HEADS = 16
B = 4
T = 2048
HD = 64
N_CORES = 8
NH_LOC = N_HEADS // 2  # heads per core
FQ = NH_LOC * HD  # 512 local features

_prog_cache = {}


def build_program(tok=T):
    """Build the single-core SPMD Bass program. tok must be a multiple of 512."""
    import concourse.mybir as mybir
    import concourse.tile as tile
    from concourse import bacc

    f32 = mybir.dt.float32
    bf16 = mybir.dt.bfloat16
    f32r = mybir.dt.float32r
    P = 128
    QC = 512  # q-chunk width
    KC = D_MODEL // P  # 8 d-model chunks
    TT = tok // P  # token tiles
    NJ = tok // QC  # q-chunks
    NDC = FQ // P  # 4 din chunks

    nc = bacc.Bacc("TRN2", target_bir_lowering=False, debug=False, num_devices=N_CORES)

    xT = nc.dram_tensor("xT", [D_MODEL, tok], f32r, kind="ExternalInput")
    wq = nc.dram_tensor("wq", [D_MODEL, FQ], f32r, kind="ExternalInput")
    wk = nc.dram_tensor("wk", [D_MODEL, FQ], f32r, kind="ExternalInput")
    wv = nc.dram_tensor("wv", [D_MODEL, FQ], f32r, kind="ExternalInput")
    wo = nc.dram_tensor("wo", [FQ, D_MODEL], f32r, kind="ExternalInput")
    y = nc.dram_tensor("y", [tok, D_MODEL], f32, kind="ExternalOutput")

    with tile.TileContext(nc) as tc:
        with (
            tc.tile_pool(name="qkt", bufs=1) as qktp,
            tc.tile_pool(name="vp", bufs=1) as vp,
            tc.tile_pool(name="mask", bufs=1) as maskp,
            # shared PSUM pools for all phases: no pool-transition barrier,
            # so the PE never sees a multi-us gap (keeps HAM at 8/8)
            tc.tile_pool(name="big", bufs=3, space="PSUM") as bigp,
            tc.tile_pool(name="small", bufs=2, space="PSUM") as smallp,
        ):
            # persistent tiles
            QKT = [qktp.tile([P, tok], f32r, tag=f"qkt{i}", name=f"qkt{i}") for i in range(8)]
            V = [vp.tile([P, NH_LOC * (HD + 1)], f32r, tag=f"v{i}", name=f"v{i}") for i in range(TT)]

            # single causal mask triangle, duplicated for the two heads:
            # mask[p, q] = 0 where q >= p else -1e30, shape [128, 2*128]
            cmask = maskp.tile([P, 2 * P], bf16, tag="cmask", name="cmask")
            nc.gpsimd.memset(cmask[:], 0.0)
            for half in (0, 1):
                nc.gpsimd.affine_select(
                    out=cmask[:, half * P : (half + 1) * P],
                    in_=cmask[:, half * P : (half + 1) * P],
                    compare_op=mybir.AluOpType.is_ge,
                    fill=-1e30,
                    base=0,
                    pattern=[[1, P]],
                    channel_multiplier=-1,
                )

            # ---------------- Phase 1: projections ----------------
            with (
                tc.tile_pool(name="xt", bufs=1) as xtp,
                tc.tile_pool(name="wst", bufs=2) as wp,
                tc.tile_pool(name="wvp", bufs=1) as wvp,
            ):
                XT = [xtp.tile([P, tok], f32r, tag=f"xt{l}", name=f"xt{l}") for l in range(KC)]
                WV = [wvp.tile([P, FQ], f32r, tag=f"wv{l}", name=f"wv{l}") for l in range(KC)]
                WSL = {}
                # DMA issue order matters on the sync queue: the first two
                # weight slices come first so the PE can start immediately
                # after the first xT tiles land
                for ft in (0, 1):
                    WSL[ft] = wp.tile([P, KC * P], f32r, tag="w", name=f"wsl{ft}")
                    nc.sync.dma_start(
                        out=WSL[ft][:].rearrange("p (l f) -> p l f", f=P),
                        in_=wq[:, ft * P : (ft + 1) * P].rearrange(
                            "(l p) f -> p l f", p=P
                        ),
                    )
                for l in range(KC):
                    nc.sync.dma_start(out=XT[l][:], in_=xT[l * P : (l + 1) * P, :])
                for l in range(KC):
                    nc.sync.dma_start(out=WV[l][:], in_=wv[l * P : (l + 1) * P, :])

                # Q^T and K^T: out[feat, tok]; lhsT = w chunk, rhs = xT chunk
                for ft in range(8):
                    wsrc = wq if ft < 4 else wk
                    fo = (ft % 4) * P
                    if ft in WSL:
                        wsl = WSL[ft]
                    else:
                        wsl = wp.tile([P, KC * P], f32r, tag="w", name=f"wsl{ft}")
                        nc.sync.dma_start(
                            out=wsl[:].rearrange("p (l f) -> p l f", f=P),
                            in_=wsrc[:, fo : fo + P].rearrange("(l p) f -> p l f", p=P),
                        )
                    qs = list(range(NJ))
                    pbs = [
                        bigp.tile([P, 2 * QC], f32, tag="big", name="pqk")
                        for _ in range((NJ + 1) // 2)
                    ]
                    half = {
                        q: pbs[q // 2][:, (q % 2) * QC : (q % 2 + 1) * QC] for q in qs
                    }
                    for l in range(KC):
                        for q in qs:
                            nc.tensor.matmul(
                                half[q],
                                wsl[:, l * P : (l + 1) * P],
                                XT[l][:, q * QC : (q + 1) * QC],
                                start=(l == 0),
                                stop=(l == KC - 1),
                            )
                    for q in qs:
                        nc.scalar.copy(QKT[ft][:, q * QC : (q + 1) * QC], half[q])

                # V: out[tok, feat]; lhsT = xT chunk, rhs = wv chunk
                for tt in range(TT):
                    nc.gpsimd.memset(V[tt][:].bitcast(f32), 1.0)
                    psv = smallp.tile([P, FQ], f32, tag="small", name="psv")
                    for l in range(KC):
                        nc.tensor.matmul(
                            psv[:],
                            XT[l][:, tt * P : (tt + 1) * P],
                            WV[l][:],
                            start=(l == 0),
                            stop=(l == KC - 1),
                        )
                    # strided copy into the 65-col head groups (ones col stays)
                    vdst = V[tt][:].rearrange("p (u c) -> p u c", c=HD + 1)[
                        :, :, 0:HD
                    ]
                    vsrc = psv[:].rearrange("p (u c) -> p u c", c=HD)
                    nc.scalar.copy(vdst, vsrc)

            # ---------------- Phase 2: causal attention ----------------
            with (
                tc.tile_pool(name="aot", bufs=1) as aotp,
                tc.tile_pool(name="exp", bufs=4) as expp,
                tc.tile_pool(name="nrm", bufs=4) as nrmp,
                tc.tile_pool(name="wop", bufs=1) as wop,
                tc.tile_pool(name="yp", bufs=4) as yp,
            ):
                AOT = [aotp.tile([P, tok], f32r, tag=f"aot{d}", name=f"aot{d}") for d in range(NDC)]
                WO = [
                    wop.tile([P, D_MODEL], f32r, tag=f"wo{d}", name=f"wo{d}") for d in range(NDC)
                ]
                for d in range(NDC):
                    nc.sync.dma_start(out=WO[d][:], in_=wo[d * P : (d + 1) * P, :])

                for j in range(NJ):
                    for hp in range(NH_LOC // 2):
                        nkt = 4 * j + 4  # k-tiles for this q-chunk
                        uA, uB = 2 * hp, 2 * hp + 1
                        us = (uA, uB)
                        pv = {u: smallp.tile([HD + 1, QC], f32, tag="small", name=f"pv{u}") for u in us}
                        for i in range(nkt):
                            # one k-tile per step; both heads packed into one
                            # [128, 1024] PSUM tile (A: cols 0:512, B: 512:1024)
                            # -> 3 steps in flight with bufs=3, and the two
                            # 64-row ST matmuls alternate PE row groups.
                            # Diagonal k-tiles (s = i - 4j >= 0) only touch
                            # q >= 128*s, so all work shrinks to the window
                            # [128*s : 512] and the mask-add reduces to one
                            # shared 128-wide causal triangle at the window
                            # start.
                            s = i - 4 * j
                            w0 = 128 * s if s >= 0 else 0
                            wn = QC - w0
                            st = bigp.tile([P, 2 * QC], f32, tag="big", name="st")
                            for idx, u in enumerate(us):
                                rs = slice(64 * (u % 2), 64 * (u % 2) + 64)
                                nc.tensor.matmul(
                                    st[:, idx * QC + w0 : (idx + 1) * QC],
                                    QKT[4 + u // 2][rs, i * P : (i + 1) * P],
                                    QKT[u // 2][rs, j * QC + w0 : (j + 1) * QC],
                                    start=True,
                                    stop=True,
                                )
                            win3 = st[:].rearrange("p (h q) -> p h q", h=2)
                            if s >= 0:
                                nc.vector.tensor_tensor(
                                    out=win3[:, :, w0 : w0 + P],
                                    in0=win3[:, :, w0 : w0 + P],
                                    in1=cmask[:].rearrange(
                                        "p (h q) -> p h q", h=2
                                    ),
                                    op=mybir.AluOpType.add,
                                )
                            e = expp.tile([P, 2 * QC], f32r, tag="e", name="e")
                            nc.scalar.activation(
                                e[:].rearrange("p (h q) -> p h q", h=2)[
                                    :, :, w0:QC
                                ],
                                win3[:, :, w0:QC],
                                mybir.ActivationFunctionType.Exp,
                                scale=0.125,
                            )
                            for idx, u in enumerate(us):
                                nc.tensor.matmul(
                                    pv[u][:, w0:QC],
                                    V[i][:, u * (HD + 1) : (u + 1) * (HD + 1)],
                                    e[:, idx * QC + w0 : (idx + 1) * QC],
                                    start=(i == 0),
                                    stop=(i == nkt - 1),
                                )
                        for u in us:
                            # copy PSUM out fast to free the pv slot, then
                            # broadcast the denominator, reciprocal on 64
                            # lanes, and normalize
                            sa = nrmp.tile([HD, QC], f32, tag="sa", name="sa")
                            nc.vector.tensor_copy(sa[:], pv[u][0:HD, :])
                            sd = nrmp.tile([1, QC], f32, tag="sd", name="sd")
                            nc.vector.tensor_copy(sd[:], pv[u][HD : HD + 1, :])
                            bc = nrmp.tile([HD, QC], f32, tag="bc", name="bc")
                            nc.gpsimd.partition_broadcast(bc[:], sd[:])
                            nc.vector.reciprocal_approx_fast(bc[:], bc[:])
                            nc.vector.tensor_tensor(
                                out=AOT[u // 2][
                                    64 * (u % 2) : 64 * (u % 2) + 64,
                                    j * QC : (j + 1) * QC,
                                ],
                                in0=sa[:],
                                in1=bc[:],
                                op=mybir.AluOpType.mult,
                            )

                # ---------------- Phase 3: output projection ----------------
                for tt in range(TT):
                    pb = bigp.tile([P, 2 * QC], f32, tag="big", name="py")
                    for d in range(NDC):
                        for h in (0, 1):
                            nc.tensor.matmul(
                                pb[:, h * QC : (h + 1) * QC],
                                AOT[d][:, tt * P : (tt + 1) * P],
                                WO[d][:, h * QC : (h + 1) * QC],
                                start=(d == 0),
                                stop=(d == NDC - 1),
                            )
                    for h in (0, 1):
                        ysb = yp.tile([P, QC], f32, tag="y")
                        nc.scalar.copy(ysb[:], pb[:, h * QC : (h + 1) * QC])
                        nc.sync.dma_start(
                            out=y[tt * P : (tt + 1) * P, h * QC : (h + 1) * QC],
                            in_=ysb[:],
                        )
    nc.compile()
    return nc


def get_program(tok=T):
    if tok not in _prog_cache:
        _prog_cache[tok] = build_program(tok)
    return _prog_cache[tok]


def make_in_maps(x, w_qkv, w_out):
    """Shard full inputs into 8 per-core input maps."""
    x = np.asarray(x, dtype=np.float32)
    w_qkv = np.asarray(w_qkv, dtype=np.float32)
    w_out = np.asarray(w_out, dtype=np.float32)
    D = D_MODEL
    xTs = [np.ascontiguousarray(x[b].T) for b in range(x.shape[0])]
    in_maps = []
    for c in range(N_CORES):
        b, hg = c // 2, c % 2
        in_maps.append(
            {
                "xT": xTs[b],
                "wq": np.ascontiguousarray(w_qkv[:, hg * FQ : (hg + 1) * FQ]),
                "wk": np.ascontiguousarray(
                    w_qkv[:, D + hg * FQ : D + (hg + 1) * FQ]
                ),
                "wv": np.ascontiguousarray(
                    w_qkv[:, 2 * D + hg * FQ : 2 * D + (hg + 1) * FQ]
                ),
                "wo": np.ascontiguousarray(w_out[hg * FQ : (hg + 1) * FQ, :]),
            }
        )
    return in_maps


_runner_cache = {}


def _make_runner(nc, n_cores=N_CORES):
    """Cached multi-core executor (same semantics as bass2jax.run_bass_via_pjrt
    for a program with no partition-id and no debug tensors, but the jitted
    callable is reusable so repeat kernel() calls don't recompile)."""
    import jax
    from jax.sharding import Mesh, PartitionSpec
    from jax.experimental.shard_map import shard_map
    import concourse.mybir as mybir
    from concourse.bass2jax import _bass_exec_p, install_neuronx_cc_hook

    install_neuronx_cc_hook()

    in_names, out_names, out_avals = [], [], []
    for alloc in nc.m.functions[0].allocations:
        if not isinstance(alloc, mybir.MemoryLocationSet):
            continue
        name = alloc.memorylocations[0].name
        if alloc.kind == "ExternalInput":
            in_names.append(name)
        elif alloc.kind == "ExternalOutput":
            out_names.append(name)
            out_avals.append(
                jax.core.ShapedArray(
                    tuple(alloc.tensor_shape), mybir.dt.np(alloc.dtype)
                )
            )
    n_params = len(out_names) and len(in_names)
    n_params = len(in_names)
    n_outs = len(out_avals)
    all_in_names = in_names + out_names

    def _body(*args):
        outs = _bass_exec_p.bind(
            *args,
            out_avals=tuple(out_avals),
            in_names=tuple(all_in_names),
            out_names=tuple(out_names),
            lowering_input_output_aliases=(),
            sim_require_finite=True,
            sim_require_nnan=True,
            nc=nc,
        )
        return tuple(outs)

    devices = jax.devices()[:n_cores]
    mesh = Mesh(np.asarray(devices), ("core",))
    donate = tuple(range(n_params, n_params + n_outs))
    sharded = jax.jit(
        shard_map(
            _body,
            mesh=mesh,
            in_specs=(PartitionSpec("core"),) * (n_params + n_outs),
            out_specs=(PartitionSpec("core"),) * n_outs,
            check_rep=False,
        ),
        donate_argnums=donate,
        keep_unused=True,
    )

    def run(in_maps):
        per_core = [[np.asarray(m[nm]) for nm in in_names] for m in in_maps]
        concat_in = [
            np.concatenate([per_core[c][i] for c in range(n_cores)], axis=0)
            for i in range(n_params)
        ]
        concat_zeros = [
            np.zeros((n_cores * a.shape[0], *a.shape[1:]), a.dtype)
            for a in out_avals
        ]
        out_arrs = sharded(*concat_in, *concat_zeros)
        return [
            {
                nm: np.asarray(out_arrs[i]).reshape(n_cores, *out_avals[i].shape)[c]
                for i, nm in enumerate(out_names)
            }
            for c in range(n_cores)
        ]

    return run


def get_runner(tok=T):
    if tok not in _runner_cache:
        _runner_cache[tok] = _make_runner(get_program(tok))
    return _runner_cache[tok]


def kernel(x, w_qkv, w_out, b_out):
    run = get_runner(T)
    in_maps = make_in_maps(x, w_qkv, w_out)
    results = run(in_maps)
    b_out = np.asarray(b_out, dtype=np.float32)
    out = np.empty((B, T, D_MODEL), dtype=np.float32)
    for b in range(B):
        out[b] = results[2 * b]["y"] + results[2 * b + 1]["y"] + b_out
    return out
